# revision 30
# baseline (speedup 1.0000x reference)
"""Trainium2 Bass kernel for the scatter_memory problem.

Math (reference):
    scores[b,m,u] = sum_d attention[b,u,d] * mem_attention[m,u,d]
    scores = where(mask, -1e30, scores) / temperature[u]
    weights = softmax(scores, axis=m)                    # [B, M, U]
    outputs[b,u,d] = sum_m weights[b,m,u] * memory[m,u,d]
    returns (outputs, weights, memory)

Sharding: over the unit axis U (64 units -> 8 units per core). Softmax is
over M, which stays local, so there are no collectives; each core reads
only its slice of the memory-bank tensors and writes its weights slice.

Per-core device program (8 units, pipelined by the Tile scheduler). All
TensorE work is bf16 (fp32 matmuls run LOW/HIGH split passes, ~3x the
cost; PE transposes are ~450ns per 128x128 tile, so both are avoided):

  A-side (weights, near-fp32 precision):
    s = hi(att)@hi(ma) + hi(att)@lo(ma) + lo(att)@hi(ma)   3 bf16 matmuls,
        exact products accumulated in fp32 PSUM (err ~2^-16)
    copy_predicated(-1e30 where mask)                      VectorE
    em = exp(s) fp32 + row-sum accumulator                 ScalarE
    w = em * (1/sum) fp32 -> HBM                           VectorE
  T-side (outputs, bf16 precision):
    sT[m,b] = hi(ma)_tile^T @ hi(att)     (instead of transposing em)
    etb = exp(sT) -> bf16                                  ScalarE
    copy_predicated(0 where maskT)                         VectorE
    out[b,d] += etb_tile^T @ mem_tile     (bf16), then * (1/sum)

Temperature is folded into att on the host (scores/t == (att/t).mem_att).
"""

import threading
from contextlib import ExitStack

import ml_dtypes
import numpy as np

import concourse.bass as bass
import concourse.mybir as mybir
import concourse.tile as tile
from concourse import bacc
from concourse.bass_utils import run_bass_kernel_spmd

B, M, U, D = 128, 4096, 64, 32
NCORES = 8
UPC = U // NCORES  # units per core = 8
NGRP = 3           # unit triples per core (3+3+2); strips at partitions 0/32/64
MC = 1024          # m-chunk size (2 PSUM banks)
NCHUNK = M // MC   # 4
NTILE = MC // 128  # 8 m-tiles of 128 per chunk
NEG = -1e30
F32 = mybir.dt.float32
BF16 = mybir.dt.bfloat16
F16 = mybir.dt.float16
U8 = mybir.dt.uint8

_BUILD_LOCK = threading.Lock()
_NC_CACHE = {}


def _build_nc():
    nc = bacc.Bacc("TRN2", target_bir_lowering=False)

    # Units are packed in triples q = u//3 on partition strips s = u%3
    # (matmul operands may only start at partition 0/32/64):
    #   att_*[32s+d, 128q+b]   = attention[b, u, d] / temp[u]
    #   ma_*[32s+d, 4096q+m]   = mem_attention[m, u, d]
    # (hi = bf16 rounding, lo = bf16 residual)
    att_hi = nc.declare_dram_parameter("att_hi", [96, NGRP * B], F16, isOutput=False)
    att_lo = nc.declare_dram_parameter("att_lo", [96, NGRP * B], F16, isOutput=False)
    ma_hi = nc.declare_dram_parameter("ma_hi", [96, NGRP * M], F16, isOutput=False)
    ma_lo = nc.declare_dram_parameter("ma_lo", [96, NGRP * M], F16, isOutput=False)
    mask8 = nc.declare_dram_parameter("mask_u8", [UPC, B, M], U8, isOutput=False)
    # maskt[u][m%128, (m//128)*128 + b] = mask[b, m, u]
    maskt = nc.declare_dram_parameter("maskt_u8", [UPC, 128, (M // 128) * B], U8, isOutput=False)
    membk = nc.declare_dram_parameter(
        "mem_bank", [UPC, 128, M // 128, D], F16, isOutput=False
    )
    w_out = nc.declare_dram_parameter("w_out", [UPC, B, M], F32, isOutput=True)
    o_out = nc.declare_dram_parameter("o_out", [B, UPC * D], F32, isOutput=True)

    AF = mybir.ActivationFunctionType

    with ExitStack() as ctx:
        tc = ctx.enter_context(tile.TileContext(nc))

        const = ctx.enter_context(tc.tile_pool(name="const", bufs=1))
        mk_pool = ctx.enter_context(tc.tile_pool(name="mk", bufs=2))
        mkt_pool = ctx.enter_context(tc.tile_pool(name="mkt", bufs=2))
        mem_pool = ctx.enter_context(tc.tile_pool(name="mem", bufs=2))
        em_pool = ctx.enter_context(tc.tile_pool(name="em", bufs=2))
        etb_pool = ctx.enter_context(tc.tile_pool(name="etb", bufs=2))
        w_pool = ctx.enter_context(tc.tile_pool(name="w", bufs=2))
        stat_pool = ctx.enter_context(tc.tile_pool(name="stat", bufs=2))
        osb_pool = ctx.enter_context(tc.tile_pool(name="osb", bufs=1))

        sp_pool = ctx.enter_context(tc.tile_pool(name="sp", bufs=2, space="PSUM"))
        st_pool = ctx.enter_context(tc.tile_pool(name="st", bufs=1, space="PSUM"))
        op_pool = ctx.enter_context(tc.tile_pool(name="op", bufs=2, space="PSUM"))

        neginf = const.tile([128, MC], F32)
        nc.gpsimd.memset(neginf[:], NEG)
        zerob = const.tile([128, MC], BF16)
        nc.gpsimd.memset(zerob[:], 0.0)
        ah = const.tile([96, NGRP * B], F16)
        nc.sync.dma_start(ah[:], att_hi[:])
        al = const.tile([96, NGRP * B], F16)
        nc.sync.dma_start(al[:], att_lo[:])
        mh = const.tile([96, NGRP * M], F16)
        ml = const.tile([96, NGRP * M], F16)
        for qd in range(NGRP):
            cols = slice(qd * M, (qd + 1) * M)
            nc.sync.dma_start(mh[:, cols], ma_hi[:, cols])
            nc.sync.dma_start(ml[:, cols], ma_lo[:, cols])
        obuf = osb_pool.tile([B, UPC * D], F32)

        for u in range(UPC):
            q, s = u // 3, u % 3
            row = slice(s * 32, (s + 1) * 32)
            ah_u = ah[row, q * B : (q + 1) * B]
            al_u = al[row, q * B : (q + 1) * B]

            mk = mk_pool.tile([B, M], U8)
            nc.sync.dma_start(mk[:], mask8[u])
            mkt = mkt_pool.tile([128, (M // 128) * B], U8)
            nc.sync.dma_start(mkt[:], maskt[u])
            mem = mem_pool.tile([128, (M // 128) * D], F16)
            nc.sync.dma_start(mem[:], membk[u].rearrange("p t d -> p (t d)"))

            em = em_pool.tile([B, M], F32)
            etb = etb_pool.tile([128, (M // 128) * B], BF16)
            sums = stat_pool.tile([B, 8], F32)

            for c in range(NCHUNK):
                # A-side: full-precision scores for the softmax weights
                sp = sp_pool.tile([B, MC], F32)
                # term-major so consecutive matmuls alternate PSUM banks
                terms = [(ah_u, mh, True, False), (ah_u, ml, False, False), (al_u, mh, False, True)]
                for lhs, rhs_t, t_start, t_stop in terms:
                    for h in range(MC // 512):
                        o0 = q * M + c * MC + h * 512
                        nc.tensor.matmul(
                            sp[:, h * 512 : (h + 1) * 512],
                            lhs,
                            rhs_t[row, o0 : o0 + 512],
                            start=t_start,
                            stop=t_stop,
                        )
                nc.vector.copy_predicated(
                    sp[:, :], mk[:, c * MC : (c + 1) * MC], neginf[:, :]
                )
                nc.scalar.activation(
                    em[:, c * MC : (c + 1) * MC],
                    sp[:, :],
                    AF.Exp,
                    accum_out=sums[:, c : c + 1],
                )

                # T-side: transposed scores -> exp -> bf16, masked to zero
                st = st_pool.tile([128, MC], F32)
                for t in range(NTILE):
                    mt = c * NTILE + t
                    o0 = q * M + mt * 128
                    nc.tensor.matmul(
                        st[:, t * 128 : (t + 1) * 128],
                        mh[row, o0 : o0 + 128],
                        ah_u,
                        start=True,
                        stop=True,
                    )
                nc.scalar.activation(
                    etb[:, c * MC : (c + 1) * MC], st[:, :], AF.Exp
                )
                nc.vector.copy_predicated(
                    etb[:, c * MC : (c + 1) * MC],
                    mkt[:, c * MC : (c + 1) * MC],
                    zerob[:, :],
                )

            nc.vector.tensor_reduce(
                sums[:, 4:5],
                sums[:, 0:NCHUNK],
                axis=mybir.AxisListType.X,
                op=mybir.AluOpType.add,
            )
            nc.vector.reciprocal(sums[:, 5:6], sums[:, 4:5])

            w = w_pool.tile([B, M], F32)
            nc.vector.tensor_scalar_mul(w[:], em[:], sums[:, 5:6])
            nc.sync.dma_start(w_out[u], w[:])

            # four interleaved accumulator slices (breaks the same-address
            # PSUM accumulation chain so consecutive matmuls can pipeline)
            ot = op_pool.tile([B, 4 * D], F32)
            for mt in range(M // 128):
                r = mt % 4
                nc.tensor.matmul(
                    ot[:, r * D : (r + 1) * D],
                    etb[:, mt * B : (mt + 1) * B],
                    mem[:, mt * D : (mt + 1) * D],
                    start=(mt == 0),
                    stop=(mt == M // 128 - 1),
                )
            osum = stat_pool.tile([B, D], F32, tag="osum")
            nc.vector.tensor_reduce(
                osum[:],
                ot[:].rearrange("b (r d) -> b d r", r=4),
                axis=mybir.AxisListType.X,
                op=mybir.AluOpType.add,
            )
            # outputs[b, d] = osum[b, d] / sum[b]
            nc.vector.tensor_scalar_mul(obuf[:, u * D : (u + 1) * D], osum[:], sums[:, 5:6])

        nc.sync.dma_start(o_out[:], obuf[:])

    nc.compile()
    return nc


def _get_nc():
    with _BUILD_LOCK:
        if "nc" not in _NC_CACHE:
            _NC_CACHE["nc"] = _build_nc()
        return _NC_CACHE["nc"]


def _prep_inputs(attention, mem_attention, memory, temperature, mask):
    """Host-side resharding to the per-core device layouts."""
    attention = np.asarray(attention, np.float32)
    mem_attention = np.asarray(mem_attention, np.float32)
    memory = np.asarray(memory, np.float32)
    temperature = np.asarray(temperature, np.float32)
    mask = np.asarray(mask)

    att = attention / temperature[None, :, None]  # fold temperature into scores
    attT = att.transpose(1, 2, 0)  # [U, D, B]
    maT = mem_attention.transpose(1, 2, 0)  # [U, D, M]
    mkT = np.ascontiguousarray(mask.transpose(2, 0, 1)).view(np.uint8)  # [U, B, M]
    # maskt[u][m%128, (m//128)*128+b] = mask[b, m, u]
    mktt = mask.transpose(2, 1, 0).reshape(U, M // 128, 128, B).transpose(0, 2, 1, 3)
    mktt = np.ascontiguousarray(mktt).view(np.uint8).reshape(U, 128, (M // 128) * B)
    # memory [M, U, D] -> [U, 128(p), M//128(t), D] with m = t*128 + p
    memT = memory.transpose(1, 0, 2).reshape(U, M // 128, 128, D).transpose(0, 2, 1, 3)
    memT = memT.astype(np.float16)

    def split(x):
        hi = x.astype(np.float16)
        lo = (x - hi.astype(np.float32)).astype(np.float16)
        return hi, lo

    in_maps = []
    for c in range(NCORES):
        sl = slice(c * UPC, (c + 1) * UPC)
        # pack unit triples: u = 3q+s -> partition strip s, column block q
        att_q = np.zeros((96, NGRP * B), np.float32)
        ma_q = np.zeros((96, NGRP * M), np.float32)
        for lu in range(UPC):
            q, st = lu // 3, lu % 3
            att_q[st * 32 : (st + 1) * 32, q * B : (q + 1) * B] = attT[c * UPC + lu]
            ma_q[st * 32 : (st + 1) * 32, q * M : (q + 1) * M] = maT[c * UPC + lu]
        ah, al = split(att_q)
        mh, ml = split(ma_q)
        in_maps.append(
            {
                "att_hi": ah,
                "att_lo": al,
                "ma_hi": mh,
                "ma_lo": ml,
                "mask_u8": np.ascontiguousarray(mkT[sl]),
                "maskt_u8": np.ascontiguousarray(mktt[sl]),
                "mem_bank": np.ascontiguousarray(memT[sl]),
            }
        )
    return in_maps, memory


def _assemble(results):
    weights = np.empty((B, M, U), np.float32)
    outputs = np.empty((B, U, D), np.float32)
    for c in range(NCORES):
        w_core = results[c]["w_out"]  # [UPC, B, M]
        weights[:, :, c * UPC : (c + 1) * UPC] = w_core.transpose(1, 2, 0)
        outputs[:, c * UPC : (c + 1) * UPC, :] = results[c]["o_out"].reshape(B, UPC, D)
    return outputs, weights


def run(attention, mem_attention, memory, temperature, mask, trace=False, **trace_kwargs):
    """Run on the 8 NeuronCores; returns ((outputs, weights, memory), BassKernelResults)."""
    in_maps, memory_np = _prep_inputs(
        attention, mem_attention, memory, temperature, mask
    )
    nc = _get_nc()
    res = run_bass_kernel_spmd(
        nc, in_maps, list(range(NCORES)), trace=trace, **trace_kwargs
    )
    outputs, weights = _assemble(res.results)
    return (outputs, weights, memory_np), res


def kernel(attention, mem_attention, memory, temperature, mask):
    out, _ = run(attention, mem_attention, memory, temperature, mask, trace=False)
    return out


# revision 33
# speedup vs baseline: 1.1749x; 1.1749x over previous
"""Trainium2 Bass kernel for the scatter_memory problem.

Math (reference):
    scores[b,m,u] = sum_d attention[b,u,d] * mem_attention[m,u,d]
    scores = where(mask, -1e30, scores) / temperature[u]
    weights = softmax(scores, axis=m)                    # [B, M, U]
    outputs[b,u,d] = sum_m weights[b,m,u] * memory[m,u,d]
    returns (outputs, weights, memory)

Sharding: over the unit axis U (64 units -> 8 units per core). Softmax is
over M, which stays local, so there are no collectives; each core reads
only its slice of the memory-bank tensors and writes its weights slice.

Per-core device program (8 units, pipelined by the Tile scheduler). All
TensorE work is fp16 at 1 cycle/row (fp32 matmuls cost 4x; PE transposes
~450ns per 128x128 tile; both avoided). att/mem_attention are sent as
fp16 (hi, lo) pairs; units are packed two per 128 partitions:
    partitions 64*(u%2) + [0:32) = hi(ma_u),  [32:64) = lo(ma_u)

  A-side (weights, ~fp32 precision — exact 4-term product):
    s = [ah;al]^T @ [mh;ml] + [al;ah]^T @ [mh;ml]   (two K=64 matmuls:
        ah@mh + al@ml + al@mh + ah@ml, fp16 products exact in fp32 PSUM)
    copy_predicated(-1e30 where mask)                      VectorE
    em = exp(s) fp32 + row-sum accumulator                 ScalarE
    w = em * (1/sum) fp32 -> HBM                           VectorE
  T-side (outputs, fp16-score precision):
    sT[m,b] = mh_tile^T @ ah      (K=32; instead of transposing em)
    etb = exp(sT) -> bf16                                  ScalarE
    copy_predicated(0 where maskT)                         VectorE
    out[b,d] += etb_tile^T @ mem_tile  (bf16/fp16), then * (1/sum)

Temperature is folded into att on the host (scores/t == (att/t).mem_att).
"""

import threading
from contextlib import ExitStack

import numpy as np

import concourse.bass as bass
import concourse.mybir as mybir
import concourse.tile as tile
from concourse import bacc
from concourse.bass_utils import run_bass_kernel_spmd

B, M, U, D = 128, 4096, 64, 32
NCORES = 8
UPC = U // NCORES  # units per core = 8
NPAIR = UPC // 2   # unit pairs per core = 4
MC = 1024          # A-side m-chunk (2 PSUM banks)
TC = 512           # T-side m-chunk (1 PSUM bank)
NCHUNK = M // MC   # 4
NEG = -1e30
F32 = mybir.dt.float32
BF16 = mybir.dt.bfloat16
F16 = mybir.dt.float16
U8 = mybir.dt.uint8

_BUILD_LOCK = threading.Lock()
_NC_CACHE = {}


def _build_nc():
    nc = bacc.Bacc("TRN2", target_bir_lowering=False)

    # ma_p[j][64*(u%2)+{0:32 hi, 32:64 lo}+d, m] = mem_attention[m, u, d], u = 2j+{0,1}
    # at1[64*(u%2)+{hi;lo}+d, 128j+b] = attention[b, u, d]/temp[u]; at2 = {lo;hi}
    ma_p = nc.declare_dram_parameter("ma_p", [NPAIR, 128, M], F16, isOutput=False)
    at1 = nc.declare_dram_parameter("at1", [128, NPAIR * B], F16, isOutput=False)
    at2 = nc.declare_dram_parameter("at2", [128, NPAIR * B], F16, isOutput=False)
    mask8 = nc.declare_dram_parameter("mask_u8", [UPC, B, M], U8, isOutput=False)
    # maskt[u][m%128, (m//128)*128 + b] = mask[b, m, u]
    maskt = nc.declare_dram_parameter(
        "maskt_u8", [UPC, 128, (M // 128) * B], U8, isOutput=False
    )
    membk = nc.declare_dram_parameter(
        "mem_bank", [UPC, 128, M // 128, D], F16, isOutput=False
    )
    w_out = nc.declare_dram_parameter("w_out", [UPC, B, M], F32, isOutput=True)
    o_out = nc.declare_dram_parameter("o_out", [B, UPC * D], F32, isOutput=True)

    AF = mybir.ActivationFunctionType

    with ExitStack() as ctx:
        tc = ctx.enter_context(tile.TileContext(nc))

        const = ctx.enter_context(tc.tile_pool(name="const", bufs=1))
        ma_pool = ctx.enter_context(tc.tile_pool(name="ma", bufs=NPAIR))
        mk_pool = ctx.enter_context(tc.tile_pool(name="mk", bufs=2))
        mkt_pool = ctx.enter_context(tc.tile_pool(name="mkt", bufs=2))
        mem_pool = ctx.enter_context(tc.tile_pool(name="mem", bufs=2))
        em_pool = ctx.enter_context(tc.tile_pool(name="em", bufs=2))
        etb_pool = ctx.enter_context(tc.tile_pool(name="etb", bufs=2))
        w_pool = ctx.enter_context(tc.tile_pool(name="w", bufs=2))
        stat_pool = ctx.enter_context(tc.tile_pool(name="stat", bufs=2))
        osb_pool = ctx.enter_context(tc.tile_pool(name="osb", bufs=1))

        sp_pool = ctx.enter_context(tc.tile_pool(name="sp", bufs=2, space="PSUM"))
        st_pool = ctx.enter_context(tc.tile_pool(name="st", bufs=3, space="PSUM"))
        op_pool = ctx.enter_context(tc.tile_pool(name="op", bufs=1, space="PSUM"))

        neginf = const.tile([128, MC], F32)
        nc.gpsimd.memset(neginf[:], NEG)
        zerob = const.tile([128, TC], BF16)
        nc.gpsimd.memset(zerob[:], 0.0)
        a1 = const.tile([128, NPAIR * B], F16)
        nc.sync.dma_start(a1[:], at1[:])
        a2 = const.tile([128, NPAIR * B], F16)
        nc.sync.dma_start(a2[:], at2[:])
        obuf = osb_pool.tile([B, UPC * D], F32)

        mas = []
        for j in range(NPAIR):
            ma = ma_pool.tile([128, M], F16)
            nc.sync.dma_start(ma[:], ma_p[j])
            mas.append(ma)

        for u in range(UPC):
            j, base = u // 2, 64 * (u % 2)
            ma = mas[j]
            k64 = slice(base, base + 64)
            k32 = slice(base, base + 32)
            acol = slice(j * B, (j + 1) * B)

            mk = mk_pool.tile([B, M], U8)
            nc.sync.dma_start(mk[:], mask8[u])
            mkt = mkt_pool.tile([128, (M // 128) * B], U8)
            nc.sync.dma_start(mkt[:], maskt[u])
            mem = mem_pool.tile([128, (M // 128) * D], F16)
            nc.sync.dma_start(mem[:], membk[u].rearrange("p t d -> p (t d)"))

            em = em_pool.tile([B, M], F32)
            etb = etb_pool.tile([128, (M // 128) * B], BF16)
            sums = stat_pool.tile([B, 8], F32)

            for c in range(NCHUNK):
                # A-side: exact scores for the softmax weights
                sp = sp_pool.tile([B, MC], F32)
                for h in range(MC // 512):
                    o0 = c * MC + h * 512
                    dst = sp[:, h * 512 : (h + 1) * 512]
                    nc.tensor.matmul(
                        dst, a1[k64, acol], ma[k64, o0 : o0 + 512], start=True, stop=False
                    )
                    nc.tensor.matmul(
                        dst, a2[k64, acol], ma[k64, o0 : o0 + 512], start=False, stop=True
                    )
                nc.vector.copy_predicated(
                    sp[:, :], mk[:, c * MC : (c + 1) * MC], neginf[:, :]
                )
                nc.scalar.activation(
                    em[:, c * MC : (c + 1) * MC],
                    sp[:, :],
                    AF.Exp,
                    accum_out=sums[:, c : c + 1],
                )

                # T-side: transposed fp16 scores -> exp -> bf16, masked to zero
                for half in range(MC // TC):
                    st = st_pool.tile([128, TC], F32)
                    for t in range(TC // 128):
                        mt = (c * MC + half * TC) // 128 + t
                        nc.tensor.matmul(
                            st[:, t * 128 : (t + 1) * 128],
                            ma[k32, mt * 128 : (mt + 1) * 128],
                            a1[k32, acol],
                            start=True,
                            stop=True,
                        )
                    tcol = slice(c * MC + half * TC, c * MC + (half + 1) * TC)
                    nc.scalar.activation(etb[:, tcol], st[:, :], AF.Exp)
                    nc.vector.copy_predicated(etb[:, tcol], mkt[:, tcol], zerob[:, :])

            nc.vector.tensor_reduce(
                sums[:, 4:5],
                sums[:, 0:NCHUNK],
                axis=mybir.AxisListType.X,
                op=mybir.AluOpType.add,
            )
            nc.vector.reciprocal(sums[:, 5:6], sums[:, 4:5])

            w = w_pool.tile([B, M], F32)
            nc.vector.tensor_scalar_mul(w[:], em[:], sums[:, 5:6])
            nc.sync.dma_start(w_out[u], w[:])

            # four interleaved accumulator slices (avoids a same-address
            # PSUM accumulation chain between consecutive matmuls)
            ot = op_pool.tile([B, 4 * D], F32)
            for mt in range(M // 128):
                r = mt % 4
                nc.tensor.matmul(
                    ot[:, r * D : (r + 1) * D],
                    etb[:, mt * B : (mt + 1) * B],
                    mem[:, mt * D : (mt + 1) * D],
                    start=(mt == 0),
                    stop=(mt == M // 128 - 1),
                )
            osum = stat_pool.tile([B, D], F32, tag="osum")
            nc.vector.tensor_reduce(
                osum[:],
                ot[:].rearrange("b (r d) -> b d r", r=4),
                axis=mybir.AxisListType.X,
                op=mybir.AluOpType.add,
            )
            # outputs[b, d] = osum[b, d] / sum[b]
            nc.vector.tensor_scalar_mul(obuf[:, u * D : (u + 1) * D], osum[:], sums[:, 5:6])

        nc.sync.dma_start(o_out[:], obuf[:])

    nc.compile()
    return nc


def _get_nc():
    with _BUILD_LOCK:
        if "nc" not in _NC_CACHE:
            _NC_CACHE["nc"] = _build_nc()
        return _NC_CACHE["nc"]


def _prep_inputs(attention, mem_attention, memory, temperature, mask):
    """Host-side resharding to the per-core device layouts."""
    attention = np.asarray(attention, np.float32)
    mem_attention = np.asarray(mem_attention, np.float32)
    memory = np.asarray(memory, np.float32)
    temperature = np.asarray(temperature, np.float32)
    mask = np.asarray(mask)

    att = attention / temperature[None, :, None]  # fold temperature into scores
    attT = att.transpose(1, 2, 0)  # [U, D, B]
    maT = mem_attention.transpose(1, 2, 0)  # [U, D, M]
    mkT = np.ascontiguousarray(mask.transpose(2, 0, 1)).view(np.uint8)  # [U, B, M]
    # maskt[u][m%128, (m//128)*128+b] = mask[b, m, u]
    mktt = mask.transpose(2, 1, 0).reshape(U, M // 128, 128, B).transpose(0, 2, 1, 3)
    mktt = np.ascontiguousarray(mktt).view(np.uint8).reshape(U, 128, (M // 128) * B)
    # memory [M, U, D] -> [U, 128(p), M//128(t), D] with m = t*128 + p
    memT = memory.transpose(1, 0, 2).reshape(U, M // 128, 128, D).transpose(0, 2, 1, 3)
    memT = memT.astype(np.float16)

    def split(x):
        hi = x.astype(np.float16)
        lo = (x - hi.astype(np.float32)).astype(np.float16)
        return hi, lo

    in_maps = []
    for c in range(NCORES):
        ma_pk = np.zeros((NPAIR, 128, M), np.float16)
        a1_pk = np.zeros((128, NPAIR * B), np.float16)
        a2_pk = np.zeros((128, NPAIR * B), np.float16)
        for lu in range(UPC):
            j, base = lu // 2, 64 * (lu % 2)
            mh, ml = split(maT[c * UPC + lu])  # [D, M]
            ah, al = split(attT[c * UPC + lu])  # [D, B]
            ma_pk[j, base : base + 32] = mh
            ma_pk[j, base + 32 : base + 64] = ml
            a1_pk[base : base + 32, j * B : (j + 1) * B] = ah
            a1_pk[base + 32 : base + 64, j * B : (j + 1) * B] = al
            a2_pk[base : base + 32, j * B : (j + 1) * B] = al
            a2_pk[base + 32 : base + 64, j * B : (j + 1) * B] = ah
        sl = slice(c * UPC, (c + 1) * UPC)
        in_maps.append(
            {
                "ma_p": ma_pk,
                "at1": a1_pk,
                "at2": a2_pk,
                "mask_u8": np.ascontiguousarray(mkT[sl]),
                "maskt_u8": np.ascontiguousarray(mktt[sl]),
                "mem_bank": np.ascontiguousarray(memT[sl]),
            }
        )
    return in_maps, memory


def _assemble(results):
    weights = np.empty((B, M, U), np.float32)
    outputs = np.empty((B, U, D), np.float32)
    for c in range(NCORES):
        w_core = results[c]["w_out"]  # [UPC, B, M]
        weights[:, :, c * UPC : (c + 1) * UPC] = w_core.transpose(1, 2, 0)
        outputs[:, c * UPC : (c + 1) * UPC, :] = results[c]["o_out"].reshape(B, UPC, D)
    return outputs, weights


def run(attention, mem_attention, memory, temperature, mask, trace=False, **trace_kwargs):
    """Run on the 8 NeuronCores; returns ((outputs, weights, memory), BassKernelResults)."""
    in_maps, memory_np = _prep_inputs(
        attention, mem_attention, memory, temperature, mask
    )
    nc = _get_nc()
    res = run_bass_kernel_spmd(
        nc, in_maps, list(range(NCORES)), trace=trace, **trace_kwargs
    )
    outputs, weights = _assemble(res.results)
    return (outputs, weights, memory_np), res


def kernel(attention, mem_attention, memory, temperature, mask):
    out, _ = run(attention, mem_attention, memory, temperature, mask, trace=False)
    return out
